# revision 26
# baseline (speedup 1.0000x reference)
"""Trainium2 Bass kernel for nn_AttentionBlock (B=16, C=512, H=W=32, 8 heads).

Data-parallel over batch: 16 batches / 8 cores = 2 per core.

v4 design (265us baseline -> ~198us): the kernel is built around a
saturated ScalarE, whose 128 softmax exps (~1.03us each over [128,1024]
PSUM tiles) are the hard floor; every other engine is scheduled into
the shadow of that stream.
  - Attention is emitted hf-outer: per (pair, half), 8 (st) units of
    [row-tiled S pair, exp]. The per-half AV accumulators are [128,512]
    (2 PSUM banks total), freeing a 3-deep [128,1024] ring for the
    S->exp pipeline (6 banks) -- deep enough to absorb cross-engine
    semaphore latency and background-allocation jitter without
    bubbling the exp stream (93%+ duty).
  - AV in fp8 DoubleRow over st-pairs (K=256): exp emits fp8 es tiles
    (bias -2 keeps exp below the fp8e4 max; folds out of softmax
    exactly), v2 is fp8 with ones=16 columns folding the softmax
    denominator into the AV matmul.
  - Denominator: accs evicted whole to bf16 (d rows ride free in the
    partition dim), then a selector-stationary PE matmul broadcasts
    them across partitions into a ps-ring slot; one reciprocal per
    (pair, half). No DRAM bounce.
  - Scalar runs ONLY [sqrt b0, sqrt b1, exp x128] (Sqrt and Exp live in
    different ACT table sets; both sqrts precede the first exp => zero
    mid-stream table swaps).
  - Front: x loads issue on 3 queues; ~3.4us of junk matmuls release
    the HAM clock gate (PE at 2.4GHz for the LN stats); LN stats via
    ones-stationary matmuls; first exp ~36us.
  - qkgen/vgen/proj/b1-xn and all pair fins are woven into per-unit
    background slots; b0's proj rides b1's attention; the tail after
    the last exp is ~15us of fin+proj+DMA.

All matmuls bf16/fp8 (fp32 PSUM). I/O: x bf16 (host-cast), out fp32.
"""

import math

import numpy as np
import ml_dtypes

import concourse.bass as bass
import concourse.bacc as bacc
import concourse.tile as tile
from concourse import mybir
from concourse.bass_utils import run_bass_kernel_spmd

P = 128
C = 512
T = 1024
N_HEADS = 8
HD = 64
B = 16
N_CORES = 8
B_LOC = B // N_CORES  # batches per core
CCH = C // P  # channel chunks of 128
EPS = 1e-5

F32 = mybir.dt.float32
BF16 = mybir.dt.bfloat16
FP8 = mybir.dt.float8e4

HALVES = ((0, slice(0, 512)), (1, slice(512, 1024)))
# fp8 es safety shift: exp(scale*s - EXPB); folds out of softmax exactly.
EXPB = 2.0
DR = mybir.MatmulPerfMode.DoubleRow


def _emit(tc, nc, pools, aps):
    mul = mybir.AluOpType.mult
    add = mybir.AluOpType.add
    sub = mybir.AluOpType.subtract

    x_d, wqk_d, wv_d, wp_d, bqk_d, bv_d, bp_d, out_d = aps
    (const, xpool, x2pool, xnpool, statp, qkpool, hpool, expp, rdsp, outp,
     psp, accp) = pools

    xv = x_d.rearrange("b (cc p) t -> b p cc t", p=P)
    ov = out_d.rearrange("b (cc p) t -> b p cc t", p=P)

    # ---- persistent tiles ----
    wqk_sb = const.tile([P, CCH, 2 * C], FP8)
    wv_sb = const.tile([P, CCH, C], FP8)
    wp_sb = const.tile([P, CCH, C], FP8)
    bqk_sb = const.tile([P, 2 * C // P], F32)
    bp_sb = const.tile([P, CCH], F32)
    bv_b = const.tile([P, C], F32)
    ones_b = const.tile([P, P], BF16)
    eps_sb = const.tile([P, 1], F32)
    nexpb_sb = const.tile([P, 1], F32)
    # d-broadcast selectors: selA row 64 -> out partitions 0:64 (h0 d,
    # read from hrawA), selB row 0 -> partitions 64:128 (h1 d from hrawB)
    selA_sb = const.tile([P, P], BF16)
    selB_sb = const.tile([P, P], BF16)
    # per-batch v2: [s-chunk partitions, st, head*128 + (data|ones)]
    # even head: v data in cols 0:64 (ones in 64:128); odd head reversed.
    v2_t = [
        const.tile([P, 8, N_HEADS * P], FP8, name=f"v2_{b}") for b in range(B_LOC)
    ]
    # LN stats live across the weave: keep them out of the stat ring
    m_t = [const.tile([P, T], BF16, name=f"m_{b}") for b in range(B_LOC)]
    rstd_t = [const.tile([P, T], F32, name=f"rstd_{b}") for b in range(B_LOC)]

    state = [dict() for _ in range(B_LOC)]

    # ---------------- const / input loads ----------------
    def emit_consts():
        nc.vector.memset(ones_b, 1.0)
        nc.vector.memset(eps_sb, EPS)
        nc.vector.memset(nexpb_sb, -EXPB)
        nc.vector.memset(selA_sb, 0.0)
        nc.vector.memset(selB_sb, 0.0)
        nc.vector.memset(selA_sb[HD : HD + 1, 0:HD], 1.0)
        nc.vector.memset(selB_sb[0:1, HD:P], 1.0)
        nc.gpsimd.dma_start(wqk_sb, wqk_d.rearrange("(cc p) o -> p cc o", p=P))
        nc.gpsimd.dma_start(wv_sb, wv_d.rearrange("(cc p) o -> p cc o", p=P))
        nc.gpsimd.dma_start(wp_sb, wp_d.rearrange("(cc p) o -> p cc o", p=P))
        nc.gpsimd.dma_start(bqk_sb, bqk_d.rearrange("(o p) -> p o", p=P))
        nc.gpsimd.dma_start(
            bv_b,
            bass.AP(tensor=bv_d.tensor, offset=bv_d.offset, ap=[[0, P]] + list(bv_d.ap)),
        )
        nc.gpsimd.dma_start(bp_sb, bp_d.rearrange("(o p) -> p o", p=P))
        for b in range(B_LOC):
            # ones = 16 everywhere; vgen evicts overwrite the data cols.
            # Full-tile memset on gpsimd: simple AP, robustly tracked.
            nc.gpsimd.memset(v2_t[b], 16.0)

    def emit_xload(b, engs):
        S = state[b]
        S["x"] = xpool.tile([P, CCH, T], BF16, tag="x", name="x_t")
        for cc in range(CCH):
            engs[cc % len(engs)].dma_start(S["x"][:, cc], xv[b, :, cc])

    # ---------------- LN stats ----------------
    def c_sq(b, cc, sc=False):
        S = state[b]
        if "x2" not in S:
            S["x2"] = x2pool.tile([P, CCH, T], BF16, tag="x2", name="x2_t")
        if sc:
            nc.scalar.activation(
                S["x2"][:, cc], S["x"][:, cc],
                mybir.ActivationFunctionType.Square)
        else:
            nc.vector.tensor_tensor(
                S["x2"][:, cc], S["x"][:, cc], S["x"][:, cc], mul)

    def c_statmm(b, cc):
        S = state[b]
        if "muB" not in S:
            S["muB"] = psp.tile([P, T], F32, tag="ps", name="ps_t")
            S["sqB"] = psp.tile([P, T], F32, tag="ps", name="ps_t")
        for _, hs in HALVES:
            nc.tensor.matmul(
                S["muB"][:, hs], ones_b, S["x"][:, cc, hs],
                start=(cc == 0), stop=(cc == CCH - 1), skip_group_check=True,
            )
        for _, hs in HALVES:
            nc.tensor.matmul(
                S["sqB"][:, hs], ones_b, S["x2"][:, cc, hs],
                start=(cc == 0), stop=(cc == CCH - 1), skip_group_check=True,
            )

    def c_statev(b):
        S = state[b]
        nc.vector.tensor_scalar_mul(m_t[b], S["muB"], 1.0 / C)
        m2 = statp.tile([P, T], BF16, tag="stat", name="stat_t")
        nc.vector.tensor_tensor(m2, m_t[b], m_t[b], mul)
        var = statp.tile([P, T], F32, tag="stat", name="stat_t")
        nc.vector.scalar_tensor_tensor(var, S["sqB"], 1.0 / C, m2, mul, sub)
        S["var"] = var
        del S["muB"], S["sqB"]

    def c_rsqrt(b):
        # sqrt on ScalarE (in-place), then the DVE fast reciprocal
        S = state[b]
        nc.scalar.activation(
            S["var"], S["var"], mybir.ActivationFunctionType.Sqrt,
            bias=eps_sb, scale=1.0,
        )
        nc.vector.reciprocal_approx_fast(rstd_t[b], S["var"])
        del S["var"]

    def c_xn_sub(b, cc):
        S = state[b]
        if "xn" not in S:
            S["xn"] = xnpool.tile([P, CCH, T], FP8, tag="xn", name="xn_t")
        t = statp.tile([P, T], BF16, tag="stat", name="stat_t")
        nc.vector.tensor_tensor(t, S["x"][:, cc], m_t[b], sub)
        S[("xt", cc)] = t

    def c_xn_mul(b, cc):
        S = state[b]
        nc.vector.tensor_tensor(S["xn"][:, cc], S[("xt", cc)], rstd_t[b], mul)
        del S[("xt", cc)]

    # ---------------- QKV ----------------
    def c_qkgen_a(b, ot):
        S = state[b]
        if "qk" not in S:
            S["qk"] = qkpool.tile([P, 8, T], BF16, tag="qk", name="qk_t")
        ps = psp.tile([P, T], F32, tag="ps", name="ps_t")
        S[("qkps", ot)] = ps
        for _, hs in HALVES:
            nc.tensor.matmul(
                ps[:, hs],
                wqk_sb[:, 0:2, ot * P : (ot + 1) * P],
                S["xn"][:, 0:2, hs],
                start=True, stop=False, perf_mode=DR, skip_group_check=True,
            )

    def c_qkgen_b(b, ot):
        S = state[b]
        ps = S[("qkps", ot)]
        for _, hs in HALVES:
            nc.tensor.matmul(
                ps[:, hs],
                wqk_sb[:, 2:4, ot * P : (ot + 1) * P],
                S["xn"][:, 2:4, hs],
                start=False, stop=True, perf_mode=DR, skip_group_check=True,
            )
        nc.vector.tensor_scalar_add(S["qk"][:, ot], ps, bqk_sb[:, ot : ot + 1])
        del S[("qkps", ot)]

    def c_qkgen(b, ot):
        c_qkgen_a(b, ot)
        c_qkgen_b(b, ot)

    def c_vgen2(b, stp):
        # two st-chunks share one PSUM tile (halves) -> one ring alloc
        S = state[b]
        ps = psp.tile([P, T], F32, tag="ps", name="ps_t")
        bvr = bv_b.rearrange("p (h c) -> p h c", c=HD)
        v2r = v2_t[b].rearrange("p st (h c) -> p st h c", c=P)
        for j in (0, 1):
            st = 2 * stp + j
            tsl = slice(st * P, (st + 1) * P)
            for i in (0, 1):
                nc.tensor.matmul(
                    ps[:, 512 * j : 512 * j + 512],
                    S["xn"][:, 2 * i : 2 * i + 2, tsl],
                    wv_sb[:, 2 * i : 2 * i + 2, :],
                    start=(i == 0), stop=(i == 1),
                    perf_mode=DR, skip_group_check=True,
                )
            pr = ps[:, 512 * j : 512 * j + 512].rearrange("p (h c) -> p h c", c=HD)
            nc.vector.tensor_tensor(
                v2r[:, st, 0::2, 0:HD], pr[:, 0::2], bvr[:, 0::2], add)
            nc.vector.tensor_tensor(
                v2r[:, st, 1::2, HD:P], pr[:, 1::2], bvr[:, 1::2], add)

    # ---------------- attention ----------------
    def c_S(b, pc, st, hf):
        # [h0 512 | h1 512] in one PSUM tile; heads run row-tiled.
        S = state[b]
        qt = S["qk"][:, 2 * pc]
        kt = S["qk"][:, 2 * pc + 1]
        hs = HALVES[hf][1]
        tsl = slice(st * P, (st + 1) * P)
        pss = psp.tile([P, T], F32, tag="ps", name="ps_t")
        for h01 in (0, 1):
            bb = slice(HD * h01, HD * h01 + HD)
            nc.tensor.matmul(
                pss[:, 512 * h01 : 512 * h01 + 512],
                kt[bb, tsl], qt[bb, hs],
                start=True, stop=True,
                tile_position=(HD * h01, 0),
            )
        S[("pss", st, hf)] = pss

    def c_exp(b, pc, st, hf):
        # es layout: [s-part, st-parity, (h0 512 | h1 512)], one per (stp, hf)
        S = state[b]
        key = ("es", st // 2, hf)
        if key not in S:
            S[key] = expp.tile([P, 2, T], FP8, tag="exp", name="exp_t")
        nc.scalar.activation(
            S[key][:, st % 2, :], S[("pss", st, hf)],
            mybir.ActivationFunctionType.Exp,
            bias=nexpb_sb, scale=0.125 / 256.0,
        )
        del S[("pss", st, hf)]

    def c_av(b, pc, hf, stp, h01):
        # fp8 DoubleRow over the st-pair (K=256) for one (half, head):
        # one [128, 512] matmul. acc tiles allocate at (stp0) after the
        # previous half's fin_evict freed the buffers.
        S = state[b]
        if stp == 0:
            S[("acc", pc, hf, h01)] = accp.tile([P, 512], F32, tag="acc",
                                                name="acc_t")
        es = S[("es", stp, hf)]
        head = 2 * pc + h01
        nc.tensor.matmul(
            S[("acc", pc, hf, h01)],
            v2_t[b][:, 2 * stp : 2 * stp + 2, head * P : (head + 1) * P],
            es[:, :, 512 * h01 : 512 * h01 + 512],
            start=(stp == 0), stop=(stp == 3),
            perf_mode=DR, skip_group_check=True,
        )
        if h01 == 1:
            del S[("es", stp, hf)]

    def c_fin_evict(b, pc, hf):
        # evict each acc fully to bf16: the d rows (partition 64 of acc0,
        # partition 0 of acc1) ride along free in the partition dim.
        S = state[b]
        hrA = rdsp.tile([P, 512], BF16, tag="hraw", name="hraw_t")
        hrB = rdsp.tile([P, 512], BF16, tag="hraw", name="hraw_t")
        nc.vector.tensor_copy(hrA, S[("acc", pc, hf, 0)])
        nc.vector.tensor_copy(hrB, S[("acc", pc, hf, 1)])
        S[("hrA", pc, hf)], S[("hrB", pc, hf)] = hrA, hrB
        del S[("acc", pc, hf, 0)], S[("acc", pc, hf, 1)]

    def c_fin_bcast(b, pc, hf):
        # PE broadcast of the d rows (sel zero-rows null the data rows)
        # into a ps-ring slot, then one reciprocal per (pair, half).
        S = state[b]
        bcT = psp.tile([P, T], F32, tag="ps", name="ps_t")
        nc.tensor.matmul(bcT[:, 0:512], selA_sb, S[("hrA", pc, hf)],
                         start=True, stop=False, skip_group_check=True)
        nc.tensor.matmul(bcT[:, 0:512], selB_sb, S[("hrB", pc, hf)],
                         start=False, stop=True, skip_group_check=True)
        rdb = rdsp.tile([P, 512], F32, tag="rdb", name="rdb_t")
        nc.vector.reciprocal_approx_fast(rdb, bcT[:, 0:512])
        S[("rdb", pc, hf)] = rdb

    def c_fin_mul(b, pc, hf, tail=False):
        S = state[b]
        if "h" not in S:
            S["h"] = hpool.tile([P, CCH, T], FP8, tag="h", name="h_t")
        hs = HALVES[hf][1]
        for h01, hr in ((0, S[("hrA", pc, hf)]), (1, S[("hrB", pc, hf)])):
            d0 = HD * h01
            eng = nc.vector if (tail and h01 == 0) else nc.gpsimd
            eng.tensor_tensor(
                S["h"][d0 : d0 + HD, pc, hs],
                hr[d0 : d0 + HD, :],
                S[("rdb", pc, hf)][d0 : d0 + HD, :],
                mul,
            )
        del S[("hrA", pc, hf)], S[("hrB", pc, hf)], S[("rdb", pc, hf)]

    # ---------------- proj + residual + out ----------------
    def c_proj_a(b, ot):
        S = state[b]
        ps = psp.tile([P, T], F32, tag="ps", name="ps_t")
        S[("pj", ot)] = ps
        for _, hs in HALVES:
            nc.tensor.matmul(
                ps[:, hs],
                wp_sb[:, 0:2, ot * P : (ot + 1) * P],
                S["h"][:, 0:2, hs],
                start=True, stop=False, perf_mode=DR, skip_group_check=True,
            )

    def c_proj_b(b, ot):
        S = state[b]
        ps = S[("pj", ot)]
        for _, hs in HALVES:
            nc.tensor.matmul(
                ps[:, hs],
                wp_sb[:, 2:4, ot * P : (ot + 1) * P],
                S["h"][:, 2:4, hs],
                start=False, stop=True, perf_mode=DR, skip_group_check=True,
            )

    def c_proj_out(b, ot, engs, tail=False):
        S = state[b]
        for hf, hs in HALVES:
            o_t = outp.tile([P, 512], F32, tag="out", name="out_t")
            nc.vector.scalar_tensor_tensor(
                o_t, S[("pj", ot)][:, hs], bp_sb[:, ot : ot + 1],
                S["x"][:, ot, hs], add, add,
            )
            for q in (0, 1):
                qs = slice(hs.start + q * 256, hs.start + q * 256 + 256)
                engs[(2 * hf + q) % len(engs)].dma_start(
                    ov[b, :, ot, qs], o_t[:, q * 256 : q * 256 + 256]
                )
        del S[("pj", ot)]

    # ================ emission schedule ================
    emit_xload(0, [nc.sync, nc.scalar, nc.gpsimd, nc.sync])
    emit_xload(1, [nc.scalar, nc.gpsimd, nc.sync, nc.scalar])
    emit_consts()
    # ~3.4us of junk matmuls releases the HAM clock gate before the LN
    # stats arrive, so the whole front runs at 2.4GHz
    warm = psp.tile([P, T], F32, tag="ps", name="ps_t")
    for _ in range(26):
        nc.tensor.matmul(warm[:, 0:P], ones_b, ones_b, start=True, stop=True,
                         skip_group_check=True)
    for cc in range(CCH):
        c_sq(0, cc)
        c_statmm(0, cc)
    c_statev(0)
    # subs need only m; they fill the DVE while ScalarE does rsqrt(0)
    for cc in range(CCH):
        c_xn_sub(0, cc)
    c_rsqrt(0)
    for cc in range(CCH):
        c_sq(1, cc)
        c_statmm(1, cc)
    c_xn_mul(0, 0)
    c_xn_mul(0, 1)
    # b1 stats eval amid the muls: rsqrt(1) gates the exp table load
    c_statev(1)
    c_rsqrt(1)  # must precede all exps (separate ACT table set)
    c_xn_mul(0, 2)
    c_xn_mul(0, 3)
    c_qkgen(0, 0)
    c_qkgen(0, 1)
    c_vgen2(0, 0)

    # Attention: hf-outer halves of 8 (st) units [S-pair, exp]. The
    # [128,512] per-half AV accumulators need only 2 PSUM banks, so the
    # S pipeline ring is 3 deep (6 banks) -- absorbing semaphore latency
    # and background-allocation jitter without bubbling the exp stream.
    # Per half: previous half's AV(stp2 h1, stp3) + fin in slots u0..u5,
    # current half's AVs in u3..u7.
    def attn_half(prev, cur, bg8):
        ex = [[] for _ in range(8)]
        if prev is not None:
            pb, ppc, phf = prev
            ex[0].append(lambda: c_av(pb, ppc, phf, 2, 1))
            ex[1].append(lambda: c_av(pb, ppc, phf, 3, 0))
            ex[2].append(lambda: c_av(pb, ppc, phf, 3, 1))
            ex[3].append(lambda: c_fin_evict(pb, ppc, phf))
            ex[4].append(lambda: c_fin_bcast(pb, ppc, phf))
            ex[5].append(lambda: c_fin_mul(pb, ppc, phf))
        cb, cpc, chf = cur
        ex[3].append(lambda: c_av(cb, cpc, chf, 0, 0))
        ex[4].append(lambda: c_av(cb, cpc, chf, 0, 1))
        ex[5].append(lambda: c_av(cb, cpc, chf, 1, 0))
        ex[6].append(lambda: c_av(cb, cpc, chf, 1, 1))
        ex[7].append(lambda: c_av(cb, cpc, chf, 2, 0))
        if bg8 is not None:
            for u in range(8):
                if bg8[u] is not None:
                    ex[u].append(bg8[u])
        for st in range(8):
            c_S(cb, cpc, st, chf)
            c_exp(cb, cpc, st, chf)
            for f in ex[st]:
                f()

    V2 = lambda b, stp: (lambda: c_vgen2(b, stp))
    Qa = lambda b, ot: (lambda: c_qkgen_a(b, ot))
    Qb = lambda b, ot: (lambda: c_qkgen_b(b, ot))
    Pa = lambda b, ot: (lambda: c_proj_a(b, ot))
    Pb = lambda b, ot: (lambda: c_proj_b(b, ot))
    XS = lambda b, cc: (lambda: c_xn_sub(b, cc))
    XM = lambda b, cc: (lambda: c_xn_mul(b, cc))
    _ = None

    def seq(*fs):
        def f():
            for g in fs:
                g()
        return f

    out_engs = [nc.sync, nc.gpsimd]
    Po = lambda b, ot: (lambda: c_proj_out(b, ot, out_engs))

    # bg[u0..u7] per half; ps-allocating chunks paced ~one per 2 units.
    halves = [(b, pc, hf) for b in (0, 1) for pc in range(4) for hf in (0, 1)]
    bg = {
        (0, 0, 0): [_, V2(0, 1), XS(1, 0), XM(1, 0),
                    V2(0, 2), XS(1, 1), V2(0, 3), XM(1, 1)],
        (0, 0, 1): [Qa(0, 2), seq(Qb(0, 2), XS(1, 2)), Qa(0, 3),
                    seq(Qb(0, 3), XM(1, 2)), _, XS(1, 3), Qa(0, 4),
                    seq(Qb(0, 4), XM(1, 3))],
        (0, 1, 0): [_, Qa(0, 5), Qb(0, 5), _, Qa(0, 6), Qb(0, 6), _, _],
        (0, 1, 1): [_, Qa(0, 7), Qb(0, 7), _, Qa(1, 0), Qb(1, 0), _, _],
        (0, 2, 0): [_, Qa(1, 1), Qb(1, 1), _, _, _, _, _],
        (0, 2, 1): [_, Qa(1, 2), Qb(1, 2), _, Qa(1, 3), Qb(1, 3), _, _],
        (0, 3, 0): [_, V2(1, 0), _, _, V2(1, 1), _, _, _],
        (0, 3, 1): [_, V2(1, 2), _, _, Qa(1, 4), Qb(1, 4), V2(1, 3), _],
        (1, 0, 0): [_, Qa(1, 5), Qb(1, 5), _, _, _, _, _],
        (1, 0, 1): [_, Qa(1, 6), Qb(1, 6), _, Qa(1, 7), Qb(1, 7), _, _],
        (1, 1, 0): [_, Pa(0, 0), Pb(0, 0), _, Po(0, 0), _, _, _],
        (1, 1, 1): [_, Pa(0, 1), Pb(0, 1), _, Po(0, 1), _, _, _],
        (1, 2, 0): [_, Pa(0, 2), Pb(0, 2), _, Po(0, 2), _, _, _],
        (1, 2, 1): [_, Pa(0, 3), Pb(0, 3), _, Po(0, 3), _, _, _],
        (1, 3, 0): [_] * 8,
        (1, 3, 1): [_] * 8,
    }
    prev = None
    for cur in halves:
        attn_half(prev, cur, bg[cur])
        prev = cur

    # --- tail: last half's AV(stp2 h1, stp3) + fin, interleaved with b1
    # proj (i0 matmuls only need h cc0/1, ready long ago) ---
    c_av(1, 3, 1, 2, 1)
    c_av(1, 3, 1, 3, 0)
    c_av(1, 3, 1, 3, 1)
    c_fin_evict(1, 3, 1)
    c_proj_a(1, 0)
    c_proj_a(1, 1)
    c_fin_bcast(1, 3, 1)
    c_fin_mul(1, 3, 1, tail=True)
    c_proj_b(1, 0)
    c_proj_b(1, 1)
    tail_engs = [nc.sync, nc.scalar, nc.gpsimd, nc.scalar]
    c_proj_out(1, 0, tail_engs, tail=True)
    c_proj_a(1, 2)
    c_proj_b(1, 2)
    c_proj_out(1, 1, tail_engs, tail=True)
    c_proj_a(1, 3)
    c_proj_b(1, 3)
    c_proj_out(1, 2, tail_engs, tail=True)
    c_proj_out(1, 3, tail_engs, tail=True)


def build_nc():
    nc = bacc.Bacc("TRN2", num_devices=N_CORES, debug=False)
    x = nc.declare_dram_parameter("x", [B_LOC, C, T], BF16, isOutput=False)
    wqk = nc.declare_dram_parameter("w_qkT", [C, 2 * C], FP8, isOutput=False)
    wv = nc.declare_dram_parameter("w_vT", [C, C], FP8, isOutput=False)
    wp = nc.declare_dram_parameter("w_projT", [C, C], FP8, isOutput=False)
    bqk = nc.declare_dram_parameter("b_qk", [2 * C], F32, isOutput=False)
    bv = nc.declare_dram_parameter("b_v", [C], F32, isOutput=False)
    bp = nc.declare_dram_parameter("b_proj", [C], F32, isOutput=False)
    out = nc.declare_dram_parameter("out", [B_LOC, C, T], F32, isOutput=True)
    aps = (x.ap(), wqk.ap(), wv.ap(), wp.ap(), bqk.ap(), bv.ap(), bp.ap(), out.ap())

    with tile.TileContext(nc) as tc:
        import contextlib

        with contextlib.ExitStack() as ctx:
            pools = (
                ctx.enter_context(tc.tile_pool(name="const", bufs=1)),
                ctx.enter_context(tc.tile_pool(name="x", bufs=2)),
                ctx.enter_context(tc.tile_pool(name="x2", bufs=2)),
                ctx.enter_context(tc.tile_pool(name="xn", bufs=2)),
                ctx.enter_context(tc.tile_pool(name="stat", bufs=6)),
                ctx.enter_context(tc.tile_pool(name="qk", bufs=2)),
                ctx.enter_context(tc.tile_pool(name="h", bufs=2)),
                ctx.enter_context(tc.tile_pool(name="exp", bufs=4)),
                ctx.enter_context(tc.tile_pool(name="rds", bufs=3)),
                ctx.enter_context(tc.tile_pool(name="out", bufs=8)),
                ctx.enter_context(tc.tile_pool(name="ps", bufs=3, space="PSUM")),
                ctx.enter_context(tc.tile_pool(name="acc", bufs=2, space="PSUM")),
            )
            _emit(tc, nc, pools, aps)
    nc.compile()
    return nc


def _host_prep(w_qkv, b_qkv, w_proj, b_proj):
    rows = np.arange(3 * C).reshape(N_HEADS, 3, HD)
    qk_order = []
    for pc in range(4):
        qk_order += list(rows[2 * pc, 0]) + list(rows[2 * pc + 1, 0])
        qk_order += list(rows[2 * pc, 1]) + list(rows[2 * pc + 1, 1])
    qk_order = np.array(qk_order)
    v_order = rows[:, 2, :].reshape(-1)
    # wqk/wv x16: keeps N(0, 1/sqrt(C))-scale weights out of the fp8e4
    # subnormal range; folded back via exp scale (qk) and v2 ones=16 (v).
    prep = {
        "w_qkT": np.ascontiguousarray(16.0 * w_qkv[qk_order].T).astype(
            ml_dtypes.float8_e4m3
        ),
        "w_vT": np.ascontiguousarray(16.0 * w_qkv[v_order].T).astype(
            ml_dtypes.float8_e4m3
        ),
        "w_projT": np.ascontiguousarray(w_proj.T).astype(ml_dtypes.float8_e4m3),
        "b_qk": np.ascontiguousarray(16.0 * b_qkv[qk_order]).astype(np.float32),
        "b_v": np.ascontiguousarray(16.0 * b_qkv[v_order]).astype(np.float32),
        "b_proj": np.ascontiguousarray(b_proj).astype(np.float32),
    }
    return prep


def _make_in_maps(x, w_qkv, b_qkv, w_proj, b_proj):
    prep = _host_prep(
        np.asarray(w_qkv, np.float32), np.asarray(b_qkv, np.float32),
        np.asarray(w_proj, np.float32), np.asarray(b_proj, np.float32),
    )
    xf = np.asarray(x, np.float32).reshape(B, C, T).astype(ml_dtypes.bfloat16)
    in_maps = []
    for core in range(N_CORES):
        m = dict(prep)
        m["x"] = np.ascontiguousarray(xf[core * B_LOC : (core + 1) * B_LOC])
        in_maps.append(m)
    return in_maps


_NC = None


def kernel(x, emb, w_qkv, b_qkv, w_proj, b_proj):
    global _NC
    x = np.asarray(x, dtype=np.float32)
    b, c, hh, ww = x.shape
    assert (b, c, hh * ww) == (B, C, T)
    if _NC is None:
        _NC = build_nc()
    in_maps = _make_in_maps(x, w_qkv, b_qkv, w_proj, b_proj)
    res = run_bass_kernel_spmd(_NC, in_maps, core_ids=list(range(N_CORES)), trace=False)
    out = np.concatenate([res.results[i]["out"] for i in range(N_CORES)], axis=0)
    return out.reshape(B, C, hh, ww).astype(np.float32)
